# revision 18
# baseline (speedup 1.0000x reference)
"""Trainium2 Bass kernel for conv1d->conv1d->LSTM(H=96)->Linear network.

Strategy (v2 — time-chunked parallel chains):
- conv1+conv2+LSTM input projection fold into one matrix (as before):
  pre_t = P @ x[t:t+5] + b_all; recurrent matmul rhs = [h (96); ones (1);
  x window (5)] so biases+input projection ride in the same matmul.
- The LSTM forget gates here are sigmoid(~N(0,0.7)), so state influence
  decays ~2^-W over W steps.  Split T=8192 into G*K chunks of L steps,
  run each chunk as an independent chain seeded with zero state W steps
  early (warmup outputs discarded).  W=24 gives ~4e-6 truncation error.
- Per core: 4 batch items (data-parallel over 8 cores) x G=2 groups x
  K=64 chains.  One "superstep" advances all K chains of a group by one
  timestep: 4 gate matmuls (bf16, moving dim 256), one sigmoid over all
  4*K*4 gate columns, cell update on DVE/Pool, tanh on ACT.
  The two groups are independent chains -> engines pipeline across them.
- tanh(g) via doubled pre-activation + sigmoid so one ACT op covers all
  four gates; (sg-0.5)*si etc. fused via scalar_tensor_tensor.
- Output projection (96->128, bias via ones-row) batched per 2
  supersteps per group; PSUM->SBUF on DVE; DMA to DRAM.
- Fully unrolled (no hardware loops, no back-edges): 88 supersteps.
- bf16 weights/state/elementwise (PSUM accumulation stays fp32):
  end-to-end rel err ~4.5e-3 (measured vs fp64 host emulation).
"""

import sys

sys.path.insert(0, "/opt/trn_rl_repo")

import numpy as np
import ml_dtypes

import concourse.bass as bass
import concourse.mybir as mybir
import concourse.tile as tile
from concourse import bacc
from concourse.bass_utils import run_bass_kernel_spmd

F32 = mybir.dt.float32
BF16 = mybir.dt.bfloat16
AFT = mybir.ActivationFunctionType
ALU = mybir.AluOpType

H = 96
BL = 4              # batch per core
NCORES = 8
T_SEQ = 8192
T_OUT = 8188

G = 3               # pipelined chain groups per core
K = 64              # chains per group
L = 43              # output steps per chain (G*K*L = 8256 >= 8188)
W = 12              # warmup steps per chain
S_RUN = L + W       # supersteps executed per group (59)
N_SLOT = S_RUN + 2  # rhs state slots (slot s+1 = h of superstep s)
C4 = K * BL         # columns per group-superstep (256)
GFD = 4 * C4        # gate columns per superstep (1024)
MPAD = 128          # gate weight columns padded 96->128 (enables FWL)


def build_program():
    nc = bacc.Bacc("TRN2", target_bir_lowering=False, debug=False)

    xt_d = nc.dram_tensor("xt", [6, G * (S_RUN + 1) * C4], BF16,
                          kind="ExternalInput")
    wcomb_d = nc.dram_tensor("wcomb", [102, 4 * MPAD], BF16,
                             kind="ExternalInput")
    lint_d = nc.dram_tensor("lint", [97, 128], BF16, kind="ExternalInput")
    out_d = nc.dram_tensor("out", [128, G * (S_RUN + 1) * C4], F32,
                           kind="ExternalOutput")

    with tile.TileContext(nc) as tc:
        with (
            tc.tile_pool(name="singles", bufs=1) as singles,
            tc.tile_pool(name="steps", bufs=3) as steps,
            tc.tile_pool(name="psum_g", bufs=3, space="PSUM") as psum_g,
            tc.tile_pool(name="psum_o", bufs=2, space="PSUM") as psum_o,
        ):
            wcomb = singles.tile([102, 4 * MPAD], BF16)
            lint = singles.tile([97, 128], BF16)
            rhs = [singles.tile([102, N_SLOT * C4], BF16, name=f"rhs{g}")
                   for g in range(G)]
            c_st = [singles.tile([H, C4], BF16, name=f"c{g}") for g in range(G)]
            t1_t = [singles.tile([H, C4], BF16, name=f"t1_{g}") for g in range(G)]
            t2_t = [singles.tile([H, C4], BF16, name=f"t2_{g}") for g in range(G)]
            tc_t = [singles.tile([H, C4], BF16, name=f"tc_{g}") for g in range(G)]

            nc.sync.dma_start(wcomb[:], wcomb_d.ap())
            nc.sync.dma_start(lint[:], lint_d.ap())
            for g in range(G):
                # x windows + ones rows, all slots (partitions 96-101);
                # slot S_RUN has ones only (read by the final out-proj).
                # Split so the first matmuls aren't gated on the bulk DMA.
                nsc = (S_RUN + 1) * C4
                head = 4 * C4
                nc.sync.dma_start(
                    rhs[g][96:102, 0:head],
                    xt_d.ap()[:, g * nsc : g * nsc + head],
                )
                nc.sync.dma_start(
                    rhs[g][96:102, head:nsc],
                    xt_d.ap()[:, g * nsc + head : (g + 1) * nsc],
                )
                nc.vector.memset(rhs[g][0:H, 0:C4], 0.0)   # h0 = 0
                nc.vector.memset(c_st[g][:], 0.0)          # c0 = 0

            for s in range(S_RUN):
                for g in range(G):
                    gp = psum_g.tile([MPAD, GFD], F32, tag="gates")
                    rhs_s = rhs[g][:, s * C4 : (s + 1) * C4]
                    for q in range(4):
                        nc.tensor.matmul(
                            gp[0:MPAD, q * C4 : (q + 1) * C4],
                            wcomb[:, q * MPAD : (q + 1) * MPAD],
                            rhs_s,
                            start=True,
                            stop=True,
                        )
                    # gate blocks: [i | f | o | g(doubled)]; rows 96-127 junk
                    sg = steps.tile([MPAD, GFD], BF16, tag="sg")
                    nc.scalar.activation(sg[:], gp[:], AFT.Sigmoid)
                    # cell state is stored HALVED (c' = c/2):
                    #   c' = t1 + t2,  t1 = (sig_g-0.5)*sig_i = i*tanh(g)/2,
                    #   t2 = sig_f * c'_prev;  tanh(c) = tanh(2*c') via scale
                    nc.gpsimd.tensor_mul(t2_t[g][:], sg[0:H, C4 : 2 * C4],
                                         c_st[g][:])
                    nc.vector.scalar_tensor_tensor(
                        t1_t[g][:], sg[0:H, 3 * C4 : 4 * C4], 0.5,
                        sg[0:H, 0:C4], op0=ALU.subtract, op1=ALU.mult,
                    )
                    nc.vector.tensor_add(c_st[g][:], t1_t[g][:], t2_t[g][:])
                    nc.scalar.activation(tc_t[g][:], c_st[g][:], AFT.Tanh,
                                         scale=2.0)
                    # h -> next slot's rhs rows
                    nc.vector.tensor_mul(
                        rhs[g][0:H, (s + 1) * C4 : (s + 2) * C4],
                        sg[0:H, 2 * C4 : 3 * C4],
                        tc_t[g][:],
                    )
                # output projection over slot pairs (slot t holds h_{t-1})
                proj = []
                if s % 2 == 1:
                    proj.append(s - 1)
                if s == S_RUN - 1:
                    proj.append(s)  # tail: covers slot S_RUN (h of last step)
                for sl0 in proj:
                    for g in range(G):
                        outp = psum_o.tile([128, 2 * C4], F32, tag="outp")
                        nc.tensor.matmul(
                            outp[:], lint[:],
                            rhs[g][0:97, sl0 * C4 : (sl0 + 2) * C4],
                            start=True, stop=True,
                        )
                        osb = steps.tile([128, 2 * C4], F32, tag="osb")
                        nc.vector.tensor_copy(osb[:], outp[:])
                        nc.sync.dma_start(
                            out_d.ap()[:, g * (S_RUN + 1) * C4 + sl0 * C4 :
                                       g * (S_RUN + 1) * C4 + (sl0 + 2) * C4],
                            osb[:],
                        )

    nc.compile()
    return nc


def fold_weights(conv1_w, conv1_b, conv2_w, conv2_b, w_ih, w_hh, b_ih, b_hh,
                 lin_w, lin_b):
    """Host-side folding (float64 for accuracy, cast to bf16 at the end)."""
    w1 = conv1_w.astype(np.float64)
    b1 = conv1_b.astype(np.float64)
    w2 = conv2_w.astype(np.float64)
    b2 = conv2_b.astype(np.float64)
    wih = w_ih.astype(np.float64)
    whh = w_hh.astype(np.float64)

    weff = np.zeros((32, 5))
    for k2 in range(3):
        for k1 in range(3):
            weff[:, k2 + k1] += w2[:, :, k2] @ w1[:, 0, k1]
    beff = w2.sum(axis=2) @ b1 + b2

    P = wih @ weff
    ball = wih @ beff + b_ih.astype(np.float64) + b_hh.astype(np.float64)

    # gate order [i, f, o, g] (torch rows are i, f, g, o)
    perm = np.r_[0:96, 96:192, 288:384, 192:288]
    wraw = np.zeros((102, 384))
    wraw[0:96] = whh.T[:, perm]
    wraw[96] = ball[perm]
    wraw[97:102] = P.T[:, perm]
    # tanh(x) = 2*sigmoid(2x)-1: double the g gate's pre-activation
    wraw[:, 3 * 96:] *= 2.0
    # pad each gate block 96 -> MPAD columns (FWL needs 128-col weights)
    wcomb = np.zeros((102, 4 * MPAD))
    for q in range(4):
        wcomb[:, q * MPAD: q * MPAD + 96] = wraw[:, q * 96: (q + 1) * 96]

    lint = np.zeros((97, 128))
    lint[0:96] = lin_w.T.astype(np.float64)
    lint[96] = lin_b.astype(np.float64)
    return (
        wcomb.astype(ml_dtypes.bfloat16),
        lint.astype(ml_dtypes.bfloat16),
    )


_prog_cache = {}


def _get_program():
    if "p" not in _prog_cache:
        _prog_cache["p"] = build_program()
    return _prog_cache["p"]


def _chain_starts():
    # chain ch covers output steps [ch*L, (ch+1)*L); starts W early (clamped)
    starts = np.maximum(np.arange(G * K) * L - W, 0)
    return starts


def run(inputs, trace=False):
    nc = _get_program()
    wcomb, lint = fold_weights(
        inputs["conv1_w"], inputs["conv1_b"], inputs["conv2_w"],
        inputs["conv2_b"], inputs["w_ih"], inputs["w_hh"], inputs["b_ih"],
        inputs["b_hh"], inputs["lin_w"], inputs["lin_b"],
    )
    x = inputs["input_data"][:, 0, :]          # [B, T]
    B = x.shape[0]
    starts = _chain_starts()                   # [G*K]
    # time index per (chain, superstep, tap): clamp OOB to a zero pad slot
    ti = starts[:, None, None] + np.arange(S_RUN)[None, :, None] \
        + np.arange(5)[None, None, :]          # [G*K, S_RUN, 5]
    ti = np.minimum(ti, T_SEQ)                 # pad col

    in_maps = []
    for c in range(NCORES):
        xs = x[c * BL: (c + 1) * BL].astype(ml_dtypes.bfloat16)  # [4, T]
        xpad = np.zeros((BL, T_SEQ + 5), ml_dtypes.bfloat16)
        xpad[:, :T_SEQ] = xs
        A = xpad[:, ti]                        # [4, G*K, S_RUN, 5]
        A = np.transpose(A, (3, 1, 2, 0))      # [5, G*K, S_RUN, 4]
        A = A.reshape(5, G, K, S_RUN, BL)
        A = np.transpose(A, (0, 1, 3, 2, 4))   # [5, G, S_RUN, K, 4]
        xt = np.zeros((6, G, S_RUN + 1, K, BL), ml_dtypes.bfloat16)
        xt[0] = 1.0
        xt[1:6, :, :S_RUN] = A
        in_maps.append({
            "xt": xt.reshape(6, -1),
            "wcomb": wcomb,
            "lint": lint,
        })
    res = run_bass_kernel_spmd(
        nc, in_maps, core_ids=list(range(NCORES)), trace=trace
    )
    full = np.zeros((T_OUT, B, 128), np.float32)
    for c in range(NCORES):
        o = res.results[c]["out"].reshape(128, G, S_RUN + 1, K, BL)
        for ch in range(G * K):
            g, j = divmod(ch, K)
            s_lo = 0 if ch == 0 else W
            t_lo = starts[ch] + s_lo
            t_hi = min(t_lo + (S_RUN - s_lo), (ch + 1) * L, T_OUT)
            if t_hi <= t_lo:
                continue
            # h of superstep s is in slot s+1
            blk = o[:, g, s_lo + 1: s_lo + 1 + (t_hi - t_lo), j, :]
            full[t_lo:t_hi, c * BL: (c + 1) * BL, :] = \
                np.transpose(blk, (1, 2, 0))
    return full, res


def kernel(**inputs):
    full, _ = run(inputs)
    return full


# revision 23
# speedup vs baseline: 1.0930x; 1.0930x over previous
"""Trainium2 Bass kernel for conv1d->conv1d->LSTM(H=96)->Linear network.

Strategy (v2 — time-chunked parallel chains):
- conv1+conv2+LSTM input projection fold into one matrix (as before):
  pre_t = P @ x[t:t+5] + b_all; recurrent matmul rhs = [h (96); ones (1);
  x window (5)] so biases+input projection ride in the same matmul.
- The LSTM forget gates here are sigmoid(~N(0,0.7)), so state influence
  decays ~2^-W over W steps.  Split T=8192 into G*K chunks of L steps,
  run each chunk as an independent chain seeded with zero state W steps
  early (warmup outputs discarded).  W=24 gives ~4e-6 truncation error.
- Per core: 4 batch items (data-parallel over 8 cores) x G=2 groups x
  K=64 chains.  One "superstep" advances all K chains of a group by one
  timestep: 4 gate matmuls (bf16, moving dim 256), one sigmoid over all
  4*K*4 gate columns, cell update on DVE/Pool, tanh on ACT.
  The two groups are independent chains -> engines pipeline across them.
- tanh(g) via doubled pre-activation + sigmoid so one ACT op covers all
  four gates; (sg-0.5)*si etc. fused via scalar_tensor_tensor.
- Output projection (96->128, bias via ones-row) batched per 2
  supersteps per group; PSUM->SBUF on DVE; DMA to DRAM.
- Fully unrolled (no hardware loops, no back-edges): 88 supersteps.
- bf16 weights/state/elementwise (PSUM accumulation stays fp32):
  end-to-end rel err ~4.5e-3 (measured vs fp64 host emulation).
"""

import sys

sys.path.insert(0, "/opt/trn_rl_repo")

import numpy as np
import ml_dtypes

import concourse.bass as bass
import concourse.mybir as mybir
import concourse.tile as tile
from concourse import bacc
from concourse.bass_utils import run_bass_kernel_spmd

F32 = mybir.dt.float32
BF16 = mybir.dt.bfloat16
AFT = mybir.ActivationFunctionType
ALU = mybir.AluOpType

H = 96
BL = 4              # batch per core
NCORES = 8
T_SEQ = 8192
T_OUT = 8188

G = 3               # pipelined chain groups per core
K = 64              # chains per group
L = 43              # output steps per chain (G*K*L = 8256 >= 8188)
W = 10              # warmup steps per chain
S_RUN = L + W       # supersteps executed per group (59)
N_SLOT = S_RUN + 2  # rhs state slots (slot s+1 = h of superstep s)
C4 = K * BL         # columns per group-superstep (256)
GFD = 4 * C4        # gate columns per superstep (1024)
MPAD = 128          # gate weight columns padded 96->128 (enables FWL)


def build_program():
    nc = bacc.Bacc("TRN2", target_bir_lowering=False, debug=False)

    xt_d = nc.dram_tensor("xt", [6, G * (S_RUN + 1) * C4], BF16,
                          kind="ExternalInput")
    wcomb_d = nc.dram_tensor("wcomb", [102, 4 * MPAD], BF16,
                             kind="ExternalInput")
    lint_d = nc.dram_tensor("lint", [97, 128], BF16, kind="ExternalInput")
    out_d = nc.dram_tensor("out", [128, G * (S_RUN + 1) * C4], BF16,
                           kind="ExternalOutput")

    with tile.TileContext(nc) as tc:
        with (
            tc.tile_pool(name="singles", bufs=1) as singles,
            tc.tile_pool(name="steps", bufs=4) as steps,
            tc.tile_pool(name="psum_g", bufs=3, space="PSUM") as psum_g,
            tc.tile_pool(name="psum_o", bufs=1, space="PSUM") as psum_o,
        ):
            wcomb = singles.tile([102, 4 * MPAD], BF16)
            lint = singles.tile([97, 128], BF16)
            rhs = [singles.tile([102, N_SLOT * C4], BF16, name=f"rhs{g}")
                   for g in range(G)]
            c_st = [singles.tile([H, C4], BF16, name=f"c{g}") for g in range(G)]

            nc.sync.dma_start(wcomb[:], wcomb_d.ap())
            nc.sync.dma_start(lint[:], lint_d.ap())
            for g in range(G):
                # x windows + ones rows, all slots (partitions 96-101);
                # slot S_RUN has ones only (read by the final out-proj).
                # Split so the first matmuls aren't gated on the bulk DMA.
                nsc = (S_RUN + 1) * C4
                head = 4 * C4
                nc.sync.dma_start(
                    rhs[g][96:102, 0:head],
                    xt_d.ap()[:, g * nsc : g * nsc + head],
                )
                nc.sync.dma_start(
                    rhs[g][96:102, head:nsc],
                    xt_d.ap()[:, g * nsc + head : (g + 1) * nsc],
                )
                nc.vector.memset(rhs[g][0:H, 0:C4], 0.0)   # h0 = 0
                nc.vector.memset(c_st[g][:], 0.0)          # c0 = 0

            for s in range(S_RUN):
                for g in range(G):
                    gp = psum_g.tile([MPAD, GFD], F32, tag="gates")
                    rhs_s = rhs[g][:, s * C4 : (s + 1) * C4]
                    for q in range(4):
                        nc.tensor.matmul(
                            gp[0:MPAD, q * C4 : (q + 1) * C4],
                            wcomb[:, q * MPAD : (q + 1) * MPAD],
                            rhs_s,
                            start=True,
                            stop=True,
                        )
                    # gate blocks: [i | f | o | g(doubled)]; rows 96-127 junk
                    sg = steps.tile([MPAD, GFD], BF16, tag="sg")
                    nc.scalar.activation(sg[:], gp[:], AFT.Sigmoid)
                    # cell state is stored HALVED (c' = c/2):
                    #   c' = t1 + t2,  t1 = (sig_g-0.5)*sig_i = i*tanh(g)/2,
                    #   t2 = sig_f * c'_prev;  tanh(c) = tanh(2*c') via scale
                    t2_t = steps.tile([H, C4], BF16, tag="t2")
                    t1_t = steps.tile([H, C4], BF16, tag="t1")
                    tc_t = steps.tile([H, C4], BF16, tag="tc")
                    nc.gpsimd.tensor_mul(t2_t[:], sg[0:H, C4 : 2 * C4],
                                         c_st[g][:])
                    nc.vector.scalar_tensor_tensor(
                        t1_t[:], sg[0:H, 3 * C4 : 4 * C4], 0.5,
                        sg[0:H, 0:C4], op0=ALU.subtract, op1=ALU.mult,
                    )
                    nc.vector.tensor_add(c_st[g][:], t1_t[:], t2_t[:])
                    nc.scalar.activation(tc_t[:], c_st[g][:], AFT.Tanh,
                                         scale=2.0)
                    # h -> next slot's rhs rows
                    nc.vector.tensor_mul(
                        rhs[g][0:H, (s + 1) * C4 : (s + 2) * C4],
                        sg[0:H, 2 * C4 : 3 * C4],
                        tc_t[:],
                    )
                # output projection, 4 slots per batch (slot t holds h_{t-1})
                proj = []
                if s % 4 == 2 and (s - 2) // 4 * 4 + 4 <= S_RUN + 1:
                    proj.append((s - 2, 4))
                if s == S_RUN - 1 and (S_RUN + 1) % 4 != 0:
                    proj.append(((S_RUN + 1) // 4 * 4, (S_RUN + 1) % 4))
                for sl0, nsl in proj:
                    for g in range(G):
                        outp = psum_o.tile([128, 4 * C4], F32, tag="outp")
                        for m0 in range(0, nsl * C4, 2 * C4):
                            mw = min(2 * C4, nsl * C4 - m0)
                            nc.tensor.matmul(
                                outp[:, m0 : m0 + mw], lint[:],
                                rhs[g][0:97,
                                      sl0 * C4 + m0 : sl0 * C4 + m0 + mw],
                                start=True, stop=True,
                            )
                        osb = steps.tile([128, 4 * C4], BF16, tag="osb")
                        nc.vector.tensor_copy(osb[:, 0 : nsl * C4],
                                              outp[:, 0 : nsl * C4])
                        nc.sync.dma_start(
                            out_d.ap()[:, g * (S_RUN + 1) * C4 + sl0 * C4 :
                                       g * (S_RUN + 1) * C4 +
                                       (sl0 + nsl) * C4],
                            osb[:, 0 : nsl * C4],
                        )

    nc.compile()
    return nc


def fold_weights(conv1_w, conv1_b, conv2_w, conv2_b, w_ih, w_hh, b_ih, b_hh,
                 lin_w, lin_b):
    """Host-side folding (float64 for accuracy, cast to bf16 at the end)."""
    w1 = conv1_w.astype(np.float64)
    b1 = conv1_b.astype(np.float64)
    w2 = conv2_w.astype(np.float64)
    b2 = conv2_b.astype(np.float64)
    wih = w_ih.astype(np.float64)
    whh = w_hh.astype(np.float64)

    weff = np.zeros((32, 5))
    for k2 in range(3):
        for k1 in range(3):
            weff[:, k2 + k1] += w2[:, :, k2] @ w1[:, 0, k1]
    beff = w2.sum(axis=2) @ b1 + b2

    P = wih @ weff
    ball = wih @ beff + b_ih.astype(np.float64) + b_hh.astype(np.float64)

    # gate order [i, f, o, g] (torch rows are i, f, g, o)
    perm = np.r_[0:96, 96:192, 288:384, 192:288]
    wraw = np.zeros((102, 384))
    wraw[0:96] = whh.T[:, perm]
    wraw[96] = ball[perm]
    wraw[97:102] = P.T[:, perm]
    # tanh(x) = 2*sigmoid(2x)-1: double the g gate's pre-activation
    wraw[:, 3 * 96:] *= 2.0
    # pad each gate block 96 -> MPAD columns (FWL needs 128-col weights)
    wcomb = np.zeros((102, 4 * MPAD))
    for q in range(4):
        wcomb[:, q * MPAD: q * MPAD + 96] = wraw[:, q * 96: (q + 1) * 96]

    lint = np.zeros((97, 128))
    lint[0:96] = lin_w.T.astype(np.float64)
    lint[96] = lin_b.astype(np.float64)
    return (
        wcomb.astype(ml_dtypes.bfloat16),
        lint.astype(ml_dtypes.bfloat16),
    )


_prog_cache = {}


def _get_program():
    if "p" not in _prog_cache:
        _prog_cache["p"] = build_program()
    return _prog_cache["p"]


def _chain_starts():
    # chain ch covers output steps [ch*L, (ch+1)*L); starts W early (clamped)
    starts = np.maximum(np.arange(G * K) * L - W, 0)
    return starts


def run(inputs, trace=False):
    nc = _get_program()
    wcomb, lint = fold_weights(
        inputs["conv1_w"], inputs["conv1_b"], inputs["conv2_w"],
        inputs["conv2_b"], inputs["w_ih"], inputs["w_hh"], inputs["b_ih"],
        inputs["b_hh"], inputs["lin_w"], inputs["lin_b"],
    )
    x = inputs["input_data"][:, 0, :]          # [B, T]
    B = x.shape[0]
    starts = _chain_starts()                   # [G*K]
    # time index per (chain, superstep, tap): clamp OOB to a zero pad slot
    ti = starts[:, None, None] + np.arange(S_RUN)[None, :, None] \
        + np.arange(5)[None, None, :]          # [G*K, S_RUN, 5]
    ti = np.minimum(ti, T_SEQ)                 # pad col

    in_maps = []
    for c in range(NCORES):
        xs = x[c * BL: (c + 1) * BL].astype(ml_dtypes.bfloat16)  # [4, T]
        xpad = np.zeros((BL, T_SEQ + 5), ml_dtypes.bfloat16)
        xpad[:, :T_SEQ] = xs
        A = xpad[:, ti]                        # [4, G*K, S_RUN, 5]
        A = np.transpose(A, (3, 1, 2, 0))      # [5, G*K, S_RUN, 4]
        A = A.reshape(5, G, K, S_RUN, BL)
        A = np.transpose(A, (0, 1, 3, 2, 4))   # [5, G, S_RUN, K, 4]
        xt = np.zeros((6, G, S_RUN + 1, K, BL), ml_dtypes.bfloat16)
        xt[0] = 1.0
        xt[1:6, :, :S_RUN] = A
        in_maps.append({
            "xt": xt.reshape(6, -1),
            "wcomb": wcomb,
            "lint": lint,
        })
    res = run_bass_kernel_spmd(
        nc, in_maps, core_ids=list(range(NCORES)), trace=trace
    )
    full = np.zeros((T_OUT, B, 128), np.float32)
    for c in range(NCORES):
        o = res.results[c]["out"].reshape(128, G, S_RUN + 1, K, BL)
        for ch in range(G * K):
            g, j = divmod(ch, K)
            s_lo = 0 if ch == 0 else W
            t_lo = starts[ch] + s_lo
            t_hi = min(t_lo + (S_RUN - s_lo), (ch + 1) * L, T_OUT)
            if t_hi <= t_lo:
                continue
            # h of superstep s is in slot s+1
            blk = o[:, g, s_lo + 1: s_lo + 1 + (t_hi - t_lo), j, :]
            full[t_lo:t_hi, c * BL: (c + 1) * BL, :] = \
                np.transpose(blk, (1, 2, 0))
    return full, res


def kernel(**inputs):
    full, _ = run(inputs)
    return full
